# revision 1
# baseline (speedup 1.0000x reference)
"""Trainium2 Bass kernel for a dense GQA transformer layer (pre-norm, SwiGLU MLP).

Full shapes: B=2, S=2048, H=2048, NH=16, NKV=8, HD=128, FF=5632, fp32 I/O.

Sharding across 8 NeuronCores (one SPMD program):
  core = (b, r) with b = core//4 (data-parallel over batch),
  r = core%4 (sequence-parallel, row-interleaved: core owns rows r::4 of
  batch b). Row interleaving makes the causal-attention work identical on
  every core, which a single SPMD program requires.
  K/V are computed for owned rows only and AllGather'ed (groups of 4).
  Everything else (QKV/O projections, softmax, MLP) is token-parallel with
  full weights per core. Host reassembles the row-interleaved outputs.

Precision: bf16 matmuls with fp32 PSUM accumulation; softmax, norms and
residuals in fp32. RMSNorm weights are folded into the following projection
weights host-side; all weights are pre-transposed host-side to [in, out].
"""

import sys

if "/opt/trn_rl_repo" not in sys.path:
    sys.path.insert(0, "/opt/trn_rl_repo")

import math
import os
import numpy as np
import ml_dtypes

import concourse.bass as bass
import concourse.bacc as bacc
import concourse.tile as tile
import concourse.mybir as mybir
from concourse.bass_utils import run_bass_kernel_spmd
from concourse.masks import make_identity

F32 = mybir.dt.float32
BF16 = mybir.dt.bfloat16
AFT = mybir.ActivationFunctionType
ALU = mybir.AluOpType

# ---- fixed problem dims ----
B, S, H = 2, 2048, 2048
NH, NKV, HD = 16, 8, 128
FF = 5632
EPS = 1e-6
NC = 8          # cores
TPG = 4         # cores per batch group (sequence-parallel ways)
P = 128         # partitions

MASK_CLAMP = -30000.0


def _build_program(S_, FF_, ext, masked, n_mask):
    """Emit the SPMD program.

    S_: sequence length, FF_: mlp width (parameterized for small-scale tests)
    ext: tuple, per q-tile number of 512-col key banks to compute
    masked: dict {(qt, bank): mask_slot_index} for banks needing a mask add
    n_mask: number of [128, 512] mask blocks in the mask input
    """
    TOWN = S_ // TPG              # tokens owned per core
    NT = TOWN // P                # q-tiles per core
    NB = S_ // 512                # key banks (512 cols each)
    HT = H // P                   # 16 H tiles
    FC = FF_ // P                 # FF tiles
    KVH = NKV
    assert len(ext) == NT
    QSCALE = 1.0 / math.sqrt(HD)

    nc = bacc.Bacc("TRN2", target_bir_lowering=False, debug=False,
                   num_devices=NC)

    # ---- I/O ----
    x_in = nc.dram_tensor("x", [NT, P, H], F32, kind="ExternalInput").ap()
    wqT = nc.dram_tensor("wqT", [H, NH * HD], BF16, kind="ExternalInput").ap()
    wkT = nc.dram_tensor("wkT", [H, NKV * HD], BF16, kind="ExternalInput").ap()
    wvT = nc.dram_tensor("wvT", [H, NKV * HD], BF16, kind="ExternalInput").ap()
    woT = nc.dram_tensor("woT", [NH * HD, H], BF16, kind="ExternalInput").ap()
    wgT = nc.dram_tensor("wgT", [H, FF_], BF16, kind="ExternalInput").ap()
    wuT = nc.dram_tensor("wuT", [H, FF_], BF16, kind="ExternalInput").ap()
    wdT = nc.dram_tensor("wdT", [FF_, H], BF16, kind="ExternalInput").ap()
    mask_in = nc.dram_tensor("mask", [max(n_mask, 1), P, 512], F32,
                             kind="ExternalInput").ap()
    y_out = nc.dram_tensor("y", [NT, P, H], F32, kind="ExternalOutput").ap()

    # ---- internal DRAM for the K/V AllGather (split in halves for overlap) ----
    KH = KVH // 2
    k_loc = [nc.dram_tensor(f"k_loc{i}", [KH, HD, NT, P], BF16).ap()
             for i in range(2)]
    v_loc = [nc.dram_tensor(f"v_loc{i}", [NT, P, KH, HD], BF16).ap()
             for i in range(2)]
    k_all = [nc.dram_tensor(f"k_all{i}", [TPG, KH, HD, NT, P], BF16).ap()
             for i in range(2)]
    v_all = [nc.dram_tensor(f"v_all{i}", [TPG, NT, P, KH, HD], BF16).ap()
             for i in range(2)]

    groups = [[g * TPG + i for i in range(TPG)] for g in range(NC // TPG)]

    from contextlib import ExitStack
    with ExitStack() as ctx:
        tc = ctx.enter_context(tile.TileContext(nc))
        pool = lambda name, bufs, **kw: ctx.enter_context(
            tc.tile_pool(name=name, bufs=bufs, **kw))
        singles = pool("ones", 1)
        resid_pool = pool("resid", NT)
        ybuf = pool("ybuf", HT)
        qT_pool = pool("qTp", NT)
        kv_pool = pool("kvbuf", 2)
        scratch_pool = pool("scratch", 1)
        ybf_pool = pool("ybfp", 1)
        pbf_pool = pool("pbf", 4)
        pT_pool = pool("pTp", 2)
        aT_pool = pool("aTp", 1)
        mT_pool = pool("mTp", FC)
        mask_pool = pool("maskp", max(n_mask, 1))
        small_pool = pool("small", 8)
        wrhs_pool = pool("wrhs", 6)
        wlhs_pool = pool("wlhs", 12)
        cpy_pool = pool("cpy", 4)
        ptr_pool = pool("ptr", 2, space="PSUM")
        pmm_pool = pool("pmm", 5, space="PSUM")
        psc_pool = pmm_pool
        pav_pool = pool("pav", 1, space="PSUM")

        ident = singles.tile([P, P], BF16)
        make_identity(nc, ident)
        eps_c = singles.tile([P, 1], F32)
        nc.vector.memset(eps_c, EPS)

        # mask blocks (fp32, resident)
        mask_sb = []
        for mi in range(n_mask):
            mt = mask_pool.tile([P, 512], F32, tag="mask")
            nc.sync.dma_start(out=mt, in_=mask_in[mi])
            mask_sb.append(mt)

        def rmsnorm_to_ybf(xt):
            sq = scratch_pool.tile([P, H], F32, tag="sq")
            ssum = small_pool.tile([P, 1], F32, tag="ss")
            nc.scalar.activation(out=sq, in_=xt, func=AFT.Square,
                                 accum_out=ssum)
            std = small_pool.tile([P, 1], F32, tag="std")
            nc.scalar.activation(out=std, in_=ssum, func=AFT.Sqrt,
                                 scale=1.0 / H, bias=eps_c)
            rstd = small_pool.tile([P, 1], F32, tag="rstd")
            nc.vector.reciprocal(rstd, std)
            ybf = ybf_pool.tile([P, H], BF16, tag="ybf")
            nc.scalar.activation(out=ybf, in_=xt, func=AFT.Copy, scale=rstd)
            return ybf

        def transpose_into(dst_tiles, ybf, qt):
            # ybf [128, H] (tokens x H) -> dst_tiles[ht][:, qt*P:...] (H x tok)
            for ht in range(HT):
                ptr = ptr_pool.tile([P, 2, P], BF16, tag="tr")
                nc.tensor.transpose(ptr[:, 0, :],
                                    ybf[:, ht * P:(ht + 1) * P], ident)
                nc.vector.tensor_copy(dst_tiles[ht][:, qt * P:(qt + 1) * P],
                                      ptr[:, 0, :])

        # Y[ht] : transposed normed activations [128 (H), TOWN] bf16
        Y = [ybuf.tile([P, TOWN], BF16, tag="y", name=f"Y{i}") for i in range(HT)]
        x_tiles = []

        # ---- stage A: load x, rmsnorm1, transpose into Y ----
        for qt in range(NT):
            xt = resid_pool.tile([P, H], F32, tag="x")
            x_tiles.append(xt)
            nc.sync.dma_start(out=xt, in_=x_in[qt])
            ybf = rmsnorm_to_ybf(xt)
            transpose_into(Y, ybf, qt)

        # ---- stage B: K/V for owned tokens, AllGather ----
        for kvp in range(KVH // 2):        # kv head pairs
            pk = [pmm_pool.tile([P, 512], F32, tag="mm", name=f"pk{j}") for j in range(2)]
            for ht in range(HT):
                wl = wlhs_pool.tile([P, 2, P], BF16, tag="wl")
                nc.sync.dma_start(
                    out=wl.rearrange("k a b -> k (a b)"),
                    in_=wkT[ht * P:(ht + 1) * P,
                            kvp * 2 * HD:(kvp + 1) * 2 * HD])
                for j in range(2):
                    nc.tensor.matmul(pk[j][:, :TOWN], lhsT=wl[:, j, :],
                                     rhs=Y[ht], start=(ht == 0),
                                     stop=(ht == HT - 1))
            for j in range(2):
                kvh = kvp * 2 + j
                kc = cpy_pool.tile([P, 512], BF16, tag="kc")
                nc.vector.tensor_copy(kc[:, :TOWN], pk[j][:, :TOWN])
                nc.sync.dma_start(
                    out=k_loc[kvh // KH][kvh % KH].rearrange(
                        "d t i -> d (t i)"),
                    in_=kc[:, :TOWN])

        for t in range(NT):
            for half in range(2):
                pv = pmm_pool.tile([P, 512], F32, tag="mm")
                for ht in range(HT):
                    wr = wrhs_pool.tile([P, 512], BF16, tag="wr")
                    nc.sync.dma_start(
                        out=wr, in_=wvT[ht * P:(ht + 1) * P,
                                        half * 512:(half + 1) * 512])
                    nc.tensor.matmul(pv, lhsT=Y[ht][:, t * P:(t + 1) * P],
                                     rhs=wr, start=(ht == 0),
                                     stop=(ht == HT - 1))
                vc = cpy_pool.tile([P, 512], BF16, tag="kc")
                nc.vector.tensor_copy(vc, pv)
                nc.sync.dma_start(
                    out=v_loc[half][t].rearrange("s k d -> s (k d)"),
                    in_=vc)

        for i in range(2):
            nc.gpsimd.collective_compute(
                "AllGather", ALU.bypass, ins=[k_loc[i].opt()],
                outs=[k_all[i].opt()], replica_groups=groups)
            nc.gpsimd.collective_compute(
                "AllGather", ALU.bypass, ins=[v_loc[i].opt()],
                outs=[v_all[i].opt()], replica_groups=groups)

        # ---- stage C: Q for owned tokens -> qT[qt] [128(d), NH, 128(tq)] ----
        qT = []
        for qt in range(NT):
            qn = scratch_pool.tile([P, NH * HD], BF16, tag="qn")
            for oc in range(NH * HD // 512):
                pq = pmm_pool.tile([P, 512], F32, tag="mm")
                for ht in range(HT):
                    wr = wrhs_pool.tile([P, 512], BF16, tag="wr")
                    nc.sync.dma_start(
                        out=wr, in_=wqT[ht * P:(ht + 1) * P,
                                        oc * 512:(oc + 1) * 512])
                    nc.tensor.matmul(pq, lhsT=Y[ht][:, qt * P:(qt + 1) * P],
                                     rhs=wr, start=(ht == 0),
                                     stop=(ht == HT - 1))
                nc.vector.tensor_copy(qn[:, oc * 512:(oc + 1) * 512], pq)
            qTt = qT_pool.tile([P, NH, P], BF16, tag="qT")
            qT.append(qTt)
            for h in range(NH):
                ptr = ptr_pool.tile([P, 2, P], BF16, tag="tr")
                nc.tensor.transpose(ptr[:, 0, :], qn[:, h * P:(h + 1) * P],
                                    ident)
                nc.vector.tensor_copy(qTt[:, h, :], ptr[:, 0, :])

        # ---- stage D: attention ----
        aT = aT_pool.tile([P, NH, TOWN], BF16, tag="aT")
        for kvh in range(KVH):
            kT_sb = kv_pool.tile([P, NB, TPG, P], BF16, tag="kT")
            v_sb = kv_pool.tile([P, NB, TPG, HD], BF16, tag="vT")
            ka, va = k_all[kvh // KH], v_all[kvh // KH]
            for o in range(TPG):
                nc.sync.dma_start(out=kT_sb[:, :, o, :], in_=ka[o, kvh % KH])
                nc.sync.dma_start(
                    out=v_sb[:, :, o, :],
                    in_=va[o].rearrange("t s k d -> s t k d")[:, :, kvh % KH, :])
            for qt in range(NT):
                nbank = ext[qt]
                p_bf = []
                for h2 in range(2):
                    h = 2 * kvh + h2
                    accs = small_pool.tile([P, NB], F32, tag="accs")
                    pb = pbf_pool.tile([P, NB * 512], BF16, tag="pb")
                    p_bf.append(pb)
                    for bi in range(nbank):
                        ps = psc_pool.tile([P, 512], F32, tag="mm")
                        nc.tensor.matmul(
                            ps, lhsT=qT[qt][:, h, :],
                            rhs=kT_sb[:, bi, :, :].rearrange(
                                "d o i -> d (o i)"),
                            start=True, stop=True)
                        mi = masked.get((qt, bi))
                        if mi is not None:
                            nc.vector.tensor_add(ps, ps, mask_sb[mi])
                        nc.scalar.activation(
                            out=pb[:, bi * 512:(bi + 1) * 512], in_=ps,
                            func=AFT.Exp, accum_out=accs[:, bi:bi + 1])
                    den = small_pool.tile([P, 1], F32, tag="den")
                    nc.vector.tensor_reduce(den, accs[:, :nbank],
                                            mybir.AxisListType.X, ALU.add)
                    rec = small_pool.tile([P, 1], F32, tag="rec")
                    nc.vector.reciprocal(rec, den)
                    nc.scalar.activation(out=pb[:, :nbank * 512],
                                         in_=pb[:, :nbank * 512],
                                         func=AFT.Copy, scale=rec)
                pav = pav_pool.tile([P, 2, P], F32, tag="av")
                for pc in range(nbank * TPG):
                    pT_ps = ptr_pool.tile([P, 2, P], BF16, tag="tr")
                    for h2 in range(2):
                        nc.tensor.transpose(
                            pT_ps[:, h2, :],
                            p_bf[h2][:, pc * P:(pc + 1) * P], ident)
                    pT_sb = pT_pool.tile([P, 2, P], BF16, tag="pT")
                    nc.vector.tensor_copy(pT_sb, pT_ps)
                    nc.tensor.matmul(
                        pav, lhsT=v_sb[:, pc // TPG, pc % TPG, :],
                        rhs=pT_sb.rearrange("s h i -> s (h i)"),
                        start=(pc == 0), stop=(pc == nbank * TPG - 1))
                nc.vector.tensor_copy(
                    aT[:, 2 * kvh:2 * kvh + 2, qt * P:(qt + 1) * P], pav)

        # ---- stage E: O projection, streamed transpose + residual into x ----
        for hcp in range(HT // 2):         # H-column pairs
            po = [pmm_pool.tile([P, 512], F32, tag="mm", name=f"po{j}") for j in range(2)]
            for h in range(NH):
                wl = wlhs_pool.tile([P, 2, P], BF16, tag="wl")
                nc.sync.dma_start(
                    out=wl.rearrange("k a b -> k (a b)"),
                    in_=woT[h * P:(h + 1) * P, hcp * 2 * P:(hcp + 1) * 2 * P])
                for j in range(2):
                    nc.tensor.matmul(po[j][:, :TOWN], lhsT=wl[:, j, :],
                                     rhs=aT[:, h, :], start=(h == 0),
                                     stop=(h == NH - 1))
            for j in range(2):
                hc = hcp * 2 + j
                oc = cpy_pool.tile([P, 512], BF16, tag="kc")
                nc.vector.tensor_copy(oc[:, :TOWN], po[j][:, :TOWN])
                for qt in range(NT):
                    ptr = ptr_pool.tile([P, 2, P], BF16, tag="tr")
                    nc.tensor.transpose(ptr[:, 0, :],
                                        oc[:, qt * P:(qt + 1) * P], ident)
                    xb = x_tiles[qt][:, hc * P:(hc + 1) * P]
                    nc.vector.tensor_add(xb, xb, ptr[:, 0, :])

        # ---- rmsnorm2 -> Y2 ----
        Y2 = [ybuf.tile([P, TOWN], BF16, tag="y", name=f"Y2_{i}") for i in range(HT)]
        for qt in range(NT):
            ybf = rmsnorm_to_ybf(x_tiles[qt])
            transpose_into(Y2, ybf, qt)

        # ---- stage F: MLP ----
        mT = []
        for fcp in range(FC // 2):         # FF-tile pairs
            pg = [pmm_pool.tile([P, 512], F32, tag="mm", name=f"pg{j}") for j in range(2)]
            for ht in range(HT):
                wl = wlhs_pool.tile([P, 2, P], BF16, tag="wl")
                nc.sync.dma_start(
                    out=wl.rearrange("k a b -> k (a b)"),
                    in_=wgT[ht * P:(ht + 1) * P, fcp * 2 * P:(fcp + 1) * 2 * P])
                for j in range(2):
                    nc.tensor.matmul(pg[j][:, :TOWN], lhsT=wl[:, j, :],
                                     rhs=Y2[ht], start=(ht == 0),
                                     stop=(ht == HT - 1))
            pu = [psc_pool.tile([P, 512], F32, tag="mm", name=f"pu{j}") for j in range(2)]
            for ht in range(HT):
                wl = wlhs_pool.tile([P, 2, P], BF16, tag="wl")
                nc.sync.dma_start(
                    out=wl.rearrange("k a b -> k (a b)"),
                    in_=wuT[ht * P:(ht + 1) * P, fcp * 2 * P:(fcp + 1) * 2 * P])
                for j in range(2):
                    nc.tensor.matmul(pu[j][:, :TOWN], lhsT=wl[:, j, :],
                                     rhs=Y2[ht], start=(ht == 0),
                                     stop=(ht == HT - 1))
            for j in range(2):
                sg = cpy_pool.tile([P, 512], BF16, tag="kc")
                nc.scalar.activation(out=sg[:, :TOWN], in_=pg[j][:, :TOWN],
                                     func=AFT.Silu)
                mt = mT_pool.tile([P, TOWN], BF16, tag="mT")
                mT.append(mt)
                nc.vector.tensor_mul(mt, sg[:, :TOWN], pu[j][:, :TOWN])

        for hcp in range(HT // 2):
            pd = [pmm_pool.tile([P, 512], F32, tag="mm", name=f"pd{j}") for j in range(2)]
            for fc in range(FC):
                wl = wlhs_pool.tile([P, 2, P], BF16, tag="wl")
                nc.sync.dma_start(
                    out=wl.rearrange("k a b -> k (a b)"),
                    in_=wdT[fc * P:(fc + 1) * P, hcp * 2 * P:(hcp + 1) * 2 * P])
                for j in range(2):
                    nc.tensor.matmul(pd[j][:, :TOWN], lhsT=wl[:, j, :],
                                     rhs=mT[fc], start=(fc == 0),
                                     stop=(fc == FC - 1))
            for j in range(2):
                hc = hcp * 2 + j
                dc = cpy_pool.tile([P, 512], BF16, tag="kc")
                nc.vector.tensor_copy(dc[:, :TOWN], pd[j][:, :TOWN])
                for qt in range(NT):
                    ptr = ptr_pool.tile([P, 2, P], BF16, tag="tr")
                    nc.tensor.transpose(ptr[:, 0, :],
                                        dc[:, qt * P:(qt + 1) * P], ident)
                    xb = x_tiles[qt][:, hc * P:(hc + 1) * P]
                    nc.vector.tensor_add(xb, xb, ptr[:, 0, :])

        for qt in range(NT):
            nc.sync.dma_start(out=y_out[qt], in_=x_tiles[qt])

    nc.compile()
    return nc


_CACHE = {}
LAST_RESULT = None


def _get_program(S_, FF_, ext, masked_items, n_mask):
    key = (S_, FF_, tuple(ext), tuple(sorted(masked_items)), n_mask)
    if key not in _CACHE:
        _CACHE[key] = _build_program(S_, FF_, tuple(ext), dict(masked_items),
                                     n_mask)
    return _CACHE[key]


def _prep_weights(q_w, k_w, v_w, o_w, gate_w, up_w, down_w, ln1_w, ln2_w):
    bf = ml_dtypes.bfloat16
    wqT = np.ascontiguousarray(
        (q_w * ln1_w[None, :]).T * (1.0 / math.sqrt(HD))).astype(bf)
    wkT = np.ascontiguousarray((k_w * ln1_w[None, :]).T).astype(bf)
    wvT = np.ascontiguousarray((v_w * ln1_w[None, :]).T).astype(bf)
    woT = np.ascontiguousarray(o_w.T).astype(bf)
    wgT = np.ascontiguousarray((gate_w * ln2_w[None, :]).T).astype(bf)
    wuT = np.ascontiguousarray((up_w * ln2_w[None, :]).T).astype(bf)
    wdT = np.ascontiguousarray(down_w.T).astype(bf)
    return wqT, wkT, wvT, woT, wgT, wuT, wdT


def _mask_structure(m, S_):
    """Derive the global (ext, masked) structure from the [S, S] mask.

    Returns per-q-tile bank extents, {(qt, bank) -> mask slot}, and the
    column-order table mapping (bank, position) -> global key column.
    """
    NT = S_ // TPG // P
    NB = S_ // 512
    col_of = np.empty((NB, 512), np.int64)
    for bi in range(NB):
        for o in range(TPG):
            col_of[bi, o * P:(o + 1) * P] = o + TPG * (P * bi + np.arange(P))
    need = np.zeros((NT, NB), bool)
    nonzero = np.zeros((NT, NB), bool)
    for r in range(TPG):
        for qt in range(NT):
            rows = r + TPG * (P * qt + np.arange(P))
            sub = m[rows]
            for bi in range(NB):
                blk = sub[:, col_of[bi]]
                need[qt, bi] |= bool((blk > MASK_CLAMP).any())
                nonzero[qt, bi] |= bool((blk < 0).any())
    ext = []
    masked = {}
    for qt in range(NT):
        e = int(np.max(np.nonzero(need[qt])[0])) + 1 if need[qt].any() else 1
        ext.append(e)
        for bi in range(e):
            if nonzero[qt, bi]:
                masked[(qt, bi)] = len(masked)
    return ext, masked, col_of


def kernel(hidden_states, attention_mask, q_w, k_w, v_w, o_w,
           gate_w, up_w, down_w, ln1_w, ln2_w):
    hidden_states = np.asarray(hidden_states, np.float32)
    m = np.maximum(np.asarray(attention_mask, np.float32)[0, 0], MASK_CLAMP)
    S_ = hidden_states.shape[1]
    FF_ = gate_w.shape[0]
    NT = S_ // TPG // P

    ext, masked, col_of = _mask_structure(m, S_)
    n_mask = len(masked)
    nc = _get_program(S_, FF_, ext, tuple(masked.items()), n_mask)

    wqT, wkT, wvT, woT, wgT, wuT, wdT = _prep_weights(
        np.asarray(q_w, np.float32), np.asarray(k_w, np.float32),
        np.asarray(v_w, np.float32), np.asarray(o_w, np.float32),
        np.asarray(gate_w, np.float32), np.asarray(up_w, np.float32),
        np.asarray(down_w, np.float32), np.asarray(ln1_w, np.float32),
        np.asarray(ln2_w, np.float32))

    in_maps = []
    for core in range(NC):
        b, r = core // TPG, core % TPG
        rows = r + TPG * np.arange(S_ // TPG)
        x_own = np.ascontiguousarray(
            hidden_states[b, rows].reshape(NT, P, H))
        mask_blocks = np.zeros((max(n_mask, 1), P, 512), np.float32)
        for (qt, bi), mi in masked.items():
            qrows = r + TPG * (P * qt + np.arange(P))
            mask_blocks[mi] = m[np.ix_(qrows, col_of[bi])]
        in_maps.append({
            "x": x_own, "mask": mask_blocks,
            "wqT": wqT, "wkT": wkT, "wvT": wvT, "woT": woT,
            "wgT": wgT, "wuT": wuT, "wdT": wdT,
        })

    res = run_bass_kernel_spmd(nc, in_maps, list(range(NC)),
                               trace=bool(os.environ.get("KERNEL_TRACE")))
    global LAST_RESULT
    LAST_RESULT = res

    out = np.empty((B, S_, H), np.float32)
    for core in range(NC):
        b, r = core // TPG, core % TPG
        rows = r + TPG * np.arange(S_ // TPG)
        out[b, rows] = res.results[core]["y"].reshape(S_ // TPG, H)
    return out



# revision 11
# speedup vs baseline: 1.0909x; 1.0909x over previous
"""Trainium2 Bass kernel for a dense GQA transformer layer (pre-norm, SwiGLU MLP).

Full shapes: B=2, S=2048, H=2048, NH=16, NKV=8, HD=128, FF=5632, fp32 I/O.

Sharding across 8 NeuronCores (one SPMD program):
  core = (b, r) with b = core//4 (data-parallel over batch),
  r = core%4 (sequence-parallel, row-interleaved: core owns rows r::4 of
  batch b). Row interleaving makes the causal-attention work identical on
  every core. K/V are computed for owned rows only and AllGather'ed
  (groups of 4). Everything else is token-parallel with full weights per
  core. Host reassembles the row-interleaved outputs.

v2 performance notes vs v1:
  - All 128x128 transposes moved off the PE onto the DMA XBAR
    (dma_start(transpose=True), 2-byte dtype).
  - Weight matmuls grouped in quads sharing one rhs stream; weights are
    host-packed so each quad is one contiguous 128KB DMA.
  - Q projection is weight-stationary and emits qT directly (no transpose).
  - Softmax normalization moved from ScalarE (ACT copy-scale pass) to
    VectorE tensor_scalar_mul.
  - K/V AllGathers kicked as soon as their halves are produced, overlapped
    with the rest of the QKV projections.
  - MLP/O weight DMAs issued from the Scalar engine queue, the rest from
    Sync, to spread HWDGE dispatch cost.

Precision: bf16 matmuls with fp32 PSUM accumulation; softmax, norms and
residuals in fp32. RMSNorm weights folded into the following projection
weights host-side; weights pre-transposed and tile-packed host-side.
"""

import sys

if "/opt/trn_rl_repo" not in sys.path:
    sys.path.insert(0, "/opt/trn_rl_repo")

import math
import os
import numpy as np
import ml_dtypes

import concourse.bass as bass
import concourse.bacc as bacc
import concourse.tile as tile
import concourse.mybir as mybir
from concourse.bass_utils import run_bass_kernel_spmd

F32 = mybir.dt.float32
BF16 = mybir.dt.bfloat16
AFT = mybir.ActivationFunctionType
ALU = mybir.AluOpType

# ---- fixed problem dims ----
B, S, H = 2, 2048, 2048
NH, NKV, HD = 16, 8, 128
FF = 5632
EPS = 1e-6
NC = 8          # cores
TPG = 4         # cores per batch group (sequence-parallel ways)
P = 128         # partitions

MASK_CLAMP = -30000.0


def _build_program(S_, FF_, ext, masked, n_mask):
    """Emit the SPMD program.

    S_: sequence length, FF_: mlp width
    ext: tuple, per q-tile number of 512-col key banks to compute
    masked: dict {(qt, bank): mask_slot_index} for banks needing a mask add
    n_mask: number of [128, 512] mask blocks in the mask input
    """
    TOWN = S_ // TPG              # tokens owned per core
    NT = TOWN // P                # q-tiles per core
    NB = S_ // 512                # key banks (512 cols each)
    HT = H // P                   # 16 H tiles
    FC = FF_ // P                 # 44 FF tiles
    FQ = FC // 4                  # 11 FF quads
    HQ = HT // 4                  # 4 H quads
    KVH = NKV
    assert len(ext) == NT

    nc = bacc.Bacc("TRN2", target_bir_lowering=False, debug=False,
                   num_devices=NC)

    # ---- I/O ----
    x_in = nc.dram_tensor("x", [NT, P, H], F32, kind="ExternalInput").ap()
    # packed weight tiles (see _prep_weights)
    wkq = nc.dram_tensor("wkq", [2, HT, P, 4, P], BF16, kind="ExternalInput").ap()
    wvp = nc.dram_tensor("wvp", [HT, P, NKV * HD], BF16, kind="ExternalInput").ap()
    wqq = nc.dram_tensor("wqq", [4, HT, P, 4, P], BF16, kind="ExternalInput").ap()
    woq = nc.dram_tensor("woq", [HQ, NH, P, 4, P], BF16, kind="ExternalInput").ap()
    wgq = nc.dram_tensor("wgq", [FQ, HT, P, 4, P], BF16, kind="ExternalInput").ap()
    wuq = nc.dram_tensor("wuq", [FQ, HT, P, 4, P], BF16, kind="ExternalInput").ap()
    wdq = nc.dram_tensor("wdq", [HQ, FC, P, 4, P], BF16, kind="ExternalInput").ap()
    mask_in = nc.dram_tensor("mask", [max(n_mask, 1), P, 512], F32,
                             kind="ExternalInput").ap()
    y_out = nc.dram_tensor("y", [NT, P, H], F32, kind="ExternalOutput").ap()

    # ---- internal DRAM for the K/V AllGather (split in halves) ----
    KH = KVH // 2                  # kv heads per half (4)
    k_loc = [nc.dram_tensor(f"k_loc{i}", [KH, HD, NT, P], BF16).ap()
             for i in range(2)]
    v_loc = [nc.dram_tensor(f"v_loc{i}", [NT, P, KH, HD], BF16).ap()
             for i in range(2)]
    k_all = [nc.dram_tensor(f"k_all{i}", [TPG, KH, HD, NT, P], BF16).ap()
             for i in range(2)]
    v_all = [nc.dram_tensor(f"v_all{i}", [TPG, NT, P, KH, HD], BF16).ap()
             for i in range(2)]

    groups = [[g * TPG + i for i in range(TPG)] for g in range(NC // TPG)]

    from contextlib import ExitStack
    with ExitStack() as ctx:
        tc = ctx.enter_context(tile.TileContext(nc))
        pool = lambda name, bufs, **kw: ctx.enter_context(
            tc.tile_pool(name=name, bufs=bufs, **kw))
        singles = pool("ones", 1)
        resid_pool = pool("resid", NT)
        ybuf = pool("ybuf", 1)          # Y then Y2 (same slot, rotated)
        qa_pool = pool("qap", 1)
        aT_pool = pool("aTp", 1)
        kv_pool = pool("kvbuf", 2)
        scratch_pool = pool("scratch", 2)
        pbf_pool = pool("pbf", 2)
        pT_pool = pool("pTp", 2)
        mT_pool = pool("mTp", FC)
        mask_pool = pool("maskp", max(n_mask, 1))
        small_pool = pool("small", 8)
        wq4_pool = pool("wq4", 10)      # quad weight tiles [P,4,P]
        wv_pool = pool("wvp_sb", 3)
        cpy_pool = pool("cpy", 4)
        xt_pool = pool("xtp", 2)        # transposed residual contributions
        ps_pool = pool("ps", 6, space="PSUM")      # [P,512] f32 accumulators
        pav_pool = pool("pav", 2, space="PSUM")    # AV accumulators

        eps_c = singles.tile([P, 1], F32)
        nc.vector.memset(eps_c, EPS)

        # mask blocks (fp32, resident)
        mask_sb = []
        for mi in range(n_mask):
            mt = mask_pool.tile([P, 512], F32, tag="mask")
            nc.sync.dma_start(out=mt, in_=mask_in[mi])
            mask_sb.append(mt)

        def rmsnorm_to_ybf(xt):
            sq = scratch_pool.tile([P, H], BF16, tag="sq", bufs=1)
            ssum = small_pool.tile([P, 1], F32, tag="ss")
            nc.scalar.activation(out=sq, in_=xt, func=AFT.Square,
                                 accum_out=ssum)
            std = small_pool.tile([P, 1], F32, tag="std")
            nc.scalar.activation(out=std, in_=ssum, func=AFT.Sqrt,
                                 scale=1.0 / H, bias=eps_c)
            rstd = small_pool.tile([P, 1], F32, tag="rstd")
            nc.vector.reciprocal(rstd, std)
            ybf = scratch_pool.tile([P, H], BF16, tag="ybf")
            nc.scalar.activation(out=ybf, in_=xt, func=AFT.Copy, scale=rstd)
            return ybf

        # Y: [128 (h within tile), HT, TOWN] bf16, transposed activations
        def build_Y(x_tiles, name):
            Y = ybuf.tile([P, HT, TOWN], BF16, tag="y", name=name)
            for qt in range(NT):
                ybf = rmsnorm_to_ybf(x_tiles[qt])
                # one XBAR call: [tok,(ht h)] -> [h, ht, tok]
                nc.sync.dma_start(
                    out=Y[:, :, qt * P:(qt + 1) * P], in_=ybf,
                    transpose=True)
            return Y

        x_tiles = []
        for qt in range(NT):
            xt = resid_pool.tile([P, H], F32, tag="x")
            x_tiles.append(xt)
            nc.sync.dma_start(out=xt, in_=x_in[qt])
        Y = build_Y(x_tiles, "Y1")

        # ---- K projection (weight-stationary, quad-grouped) ----
        def k_half(kvq):
            pk = [ps_pool.tile([P, 512], F32, tag="mm", name=f"pk{kvq}_{j}")
                  for j in range(4)]
            for ht in range(HT):
                wl = wq4_pool.tile([P, 4, P], BF16, tag="w4")
                nc.sync.dma_start(out=wl.rearrange("k a b -> k (a b)"),
                                  in_=wkq[kvq, ht].rearrange("k a b -> k (a b)"))
                for j in range(4):
                    nc.tensor.matmul(pk[j][:, :TOWN], lhsT=wl[:, j, :],
                                     rhs=Y[:, ht, :], start=(ht == 0),
                                     stop=(ht == HT - 1))
            for j in range(4):
                kc = cpy_pool.tile([P, 512], BF16, tag="kc")
                nc.vector.tensor_copy(kc[:, :TOWN], pk[j][:, :TOWN])
                nc.sync.dma_start(
                    out=k_loc[kvq][j].rearrange("d t i -> d (t i)"),
                    in_=kc[:, :TOWN])

        k_half(0)

        # ---- V projection (Y-stationary): out [tok, kv_heads*hd] ----
        for t in range(NT):
            pv = [ps_pool.tile([P, 512], F32, tag="mm", name=f"pv{t}_{h}")
                  for h in range(2)]
            for ht in range(HT):
                wv = wv_pool.tile([P, NKV * HD], BF16, tag="wv")
                nc.sync.dma_start(out=wv, in_=wvp[ht])
                for h in range(2):
                    nc.tensor.matmul(pv[h], lhsT=Y[:, ht, t * P:(t + 1) * P],
                                     rhs=wv[:, h * 512:(h + 1) * 512],
                                     start=(ht == 0), stop=(ht == HT - 1))
            for h in range(2):
                vc = cpy_pool.tile([P, 512], BF16, tag="kc")
                nc.vector.tensor_copy(vc, pv[h])
                nc.sync.dma_start(
                    out=v_loc[h][t].rearrange("s k d -> s (k d)"), in_=vc)

        nc.gpsimd.collective_compute(
            "AllGather", ALU.bypass, ins=[k_loc[0].opt()],
            outs=[k_all[0].opt()], replica_groups=groups)
        nc.gpsimd.collective_compute(
            "AllGather", ALU.bypass, ins=[v_loc[0].opt()],
            outs=[v_all[0].opt()], replica_groups=groups)

        k_half(1)

        nc.gpsimd.collective_compute(
            "AllGather", ALU.bypass, ins=[k_loc[1].opt()],
            outs=[k_all[1].opt()], replica_groups=groups)
        nc.gpsimd.collective_compute(
            "AllGather", ALU.bypass, ins=[v_loc[1].opt()],
            outs=[v_all[1].opt()], replica_groups=groups)

        # ---- Q projection (weight-stationary -> qT directly) ----
        qT = qa_pool.tile([P, NH, TOWN], BF16, tag="qT")
        for qq in range(4):
            pq = [ps_pool.tile([P, 512], F32, tag="mm", name=f"pq{qq}_{j}")
                  for j in range(4)]
            for ht in range(HT):
                wl = wq4_pool.tile([P, 4, P], BF16, tag="w4")
                nc.sync.dma_start(out=wl.rearrange("k a b -> k (a b)"),
                                  in_=wqq[qq, ht].rearrange("k a b -> k (a b)"))
                for j in range(4):
                    nc.tensor.matmul(pq[j][:, :TOWN], lhsT=wl[:, j, :],
                                     rhs=Y[:, ht, :], start=(ht == 0),
                                     stop=(ht == HT - 1))
            for j in range(4):
                nc.vector.tensor_copy(qT[:, qq * 4 + j, :],
                                      pq[j][:, :TOWN])

        # ---- attention ----
        aT = aT_pool.tile([P, NH, TOWN], BF16, tag="aT")
        for kvh in range(KVH):
            kT_sb = kv_pool.tile([P, NB, TPG, P], BF16, tag="kT")
            v_sb = kv_pool.tile([P, NB, TPG, HD], BF16, tag="vT")
            ka, va = k_all[kvh // KH], v_all[kvh // KH]
            for o in range(TPG):
                nc.sync.dma_start(out=kT_sb[:, :, o, :], in_=ka[o, kvh % KH])
                nc.sync.dma_start(
                    out=v_sb[:, :, o, :],
                    in_=va[o].rearrange("t s k d -> s t k d")[:, :, kvh % KH, :])
            for qt in range(NT):
                nbank = ext[qt]
                # pT tile: [key, pc, h2, tok] bf16
                pTt = pT_pool.tile([P, NB * TPG, 2, P], BF16, tag="pT")
                for h2 in range(2):
                    h = 2 * kvh + h2
                    accs = small_pool.tile([P, NB], F32, tag="accs")
                    pb = pbf_pool.tile([P, NB * 512], BF16, tag="pb")
                    for bi in range(nbank):
                        psc = ps_pool.tile([P, 512], F32, tag="mm")
                        nc.tensor.matmul(
                            psc, lhsT=qT[:, h, qt * P:(qt + 1) * P],
                            rhs=kT_sb[:, bi, :, :].rearrange(
                                "d o i -> d (o i)"),
                            start=True, stop=True)
                        mi = masked.get((qt, bi))
                        if mi is not None:
                            nc.vector.tensor_add(psc, psc, mask_sb[mi])
                        nc.scalar.activation(
                            out=pb[:, bi * 512:(bi + 1) * 512], in_=psc,
                            func=AFT.Exp, accum_out=accs[:, bi:bi + 1])
                    den = small_pool.tile([P, 1], F32, tag="den")
                    nc.vector.tensor_reduce(den, accs[:, :nbank],
                                            mybir.AxisListType.X, ALU.add)
                    rec = small_pool.tile([P, 1], F32, tag="rec")
                    nc.vector.reciprocal(rec, den)
                    nc.vector.tensor_scalar_mul(pb[:, :nbank * 512],
                                                pb[:, :nbank * 512], rec)
                    # XBAR: [tok, (pc key)] -> [key, pc, tok]
                    nc.sync.dma_start(
                        out=pTt[:, :nbank * TPG, h2, :],
                        in_=pb[:, :nbank * 512], transpose=True)
                pav = pav_pool.tile([P, 2, P], F32, tag="av",
                                    padded_shape=[P, 2, 256])
                for pc in range(nbank * TPG):
                    nc.tensor.matmul(
                        pav, lhsT=v_sb[:, pc // TPG, pc % TPG, :],
                        rhs=pTt[:, pc, :, :].rearrange("s h i -> s (h i)"),
                        start=(pc == 0), stop=(pc == nbank * TPG - 1))
                nc.vector.tensor_copy(
                    aT[:, 2 * kvh:2 * kvh + 2, qt * P:(qt + 1) * P], pav)

        # ---- O projection + residual ----
        def out_proj_pass(w_dram, rhs_fn, n_acc, hq):
            """One quad accumulation pass producing [Hcol(4x128), TOWN] f32."""
            po = [ps_pool.tile([P, 512], F32, tag="mm", name=f"po{hq}_{j}")
                  for j in range(4)]
            for a in range(n_acc):
                wl = wq4_pool.tile([P, 4, P], BF16, tag="w4")
                nc.scalar.dma_start(
                    out=wl.rearrange("k a b -> k (a b)"),
                    in_=w_dram[hq, a].rearrange("k a b -> k (a b)"))
                rhs = rhs_fn(a)
                for j in range(4):
                    nc.tensor.matmul(po[j][:, :TOWN], lhsT=wl[:, j, :],
                                     rhs=rhs, start=(a == 0),
                                     stop=(a == n_acc - 1))
            # transpose back into residual: per j one XBAR + NT adds
            for j in range(4):
                dc = cpy_pool.tile([P, 512], BF16, tag="kc")
                nc.vector.tensor_copy(dc[:, :TOWN], po[j][:, :TOWN])
                xT = xt_pool.tile([P, NT, P], BF16, tag="xT")
                nc.sync.dma_start(out=xT[:, :NT, :], in_=dc[:, :TOWN],
                                  transpose=True)
                hc = hq * 4 + j
                for qt in range(NT):
                    xb = x_tiles[qt][:, hc * P:(hc + 1) * P]
                    nc.vector.tensor_add(xb, xb, xT[:, qt, :])

        for hq in range(HQ):
            out_proj_pass(woq, lambda h: aT[:, h, :], NH, hq)

        # ---- rmsnorm2 -> Y2 ----
        Y2 = build_Y(x_tiles, "Y2")

        # ---- MLP gate/up ----
        mT = []
        for fq in range(FQ):
            pg = [ps_pool.tile([P, 512], F32, tag="mm", name=f"pg{fq}_{j}")
                  for j in range(4)]
            for ht in range(HT):
                wl = wq4_pool.tile([P, 4, P], BF16, tag="w4")
                nc.scalar.dma_start(
                    out=wl.rearrange("k a b -> k (a b)"),
                    in_=wgq[fq, ht].rearrange("k a b -> k (a b)"))
                for j in range(4):
                    nc.tensor.matmul(pg[j][:, :TOWN], lhsT=wl[:, j, :],
                                     rhs=Y2[:, ht, :], start=(ht == 0),
                                     stop=(ht == HT - 1))
            sg = [cpy_pool.tile([P, 512], BF16, tag="kc", name=f"sg{fq}_{j}")
                  for j in range(4)]
            for j in range(4):
                nc.scalar.activation(out=sg[j][:, :TOWN], in_=pg[j][:, :TOWN],
                                     func=AFT.Silu)
            pu = [ps_pool.tile([P, 512], F32, tag="mm", name=f"pu{fq}_{j}")
                  for j in range(4)]
            for ht in range(HT):
                wl = wq4_pool.tile([P, 4, P], BF16, tag="w4")
                nc.scalar.dma_start(
                    out=wl.rearrange("k a b -> k (a b)"),
                    in_=wuq[fq, ht].rearrange("k a b -> k (a b)"))
                for j in range(4):
                    nc.tensor.matmul(pu[j][:, :TOWN], lhsT=wl[:, j, :],
                                     rhs=Y2[:, ht, :], start=(ht == 0),
                                     stop=(ht == HT - 1))
            for j in range(4):
                mt = mT_pool.tile([P, TOWN], BF16, tag="mT")
                mT.append(mt)
                nc.vector.tensor_mul(mt, sg[j][:, :TOWN], pu[j][:, :TOWN])

        # ---- MLP down + residual ----
        for hq in range(HQ):
            out_proj_pass(wdq, lambda fc: mT[fc], FC, hq)

        for qt in range(NT):
            nc.sync.dma_start(out=y_out[qt], in_=x_tiles[qt])

    nc.compile()
    return nc


_CACHE = {}
LAST_RESULT = None


def _get_program(S_, FF_, ext, masked_items, n_mask):
    key = (S_, FF_, tuple(ext), tuple(sorted(masked_items)), n_mask)
    if key not in _CACHE:
        _CACHE[key] = _build_program(S_, FF_, tuple(ext), dict(masked_items),
                                     n_mask)
    return _CACHE[key]


def _quad_pack(wT, n_outer, inner_tiles):
    """[K, N] -> [n_outer, inner_tiles, 128, 4, 128] quad tiles.

    outer o covers output cols o*512..(o+1)*512, inner a covers input rows
    a*128..(a+1)*128.
    """
    K, N = wT.shape
    assert K == inner_tiles * P and N == n_outer * 512
    w = wT.reshape(inner_tiles, P, n_outer, 4, P)
    return np.ascontiguousarray(w.transpose(2, 0, 1, 3, 4))


def _prep_weights(q_w, k_w, v_w, o_w, gate_w, up_w, down_w, ln1_w, ln2_w):
    bf = ml_dtypes.bfloat16
    HT = H // P
    FC = FF // P
    wqT = (q_w * ln1_w[None, :]).T * (1.0 / math.sqrt(HD))   # [H, NH*HD]
    wkT = (k_w * ln1_w[None, :]).T                           # [H, NKV*HD]
    wvT = (v_w * ln1_w[None, :]).T
    woT = o_w.T                                              # [NH*HD, H]
    wgT = (gate_w * ln2_w[None, :]).T                        # [H, FF]
    wuT = (up_w * ln2_w[None, :]).T
    wdT = down_w.T                                           # [FF, H]
    wkq = _quad_pack(wkT, 2, HT).astype(bf)
    wvp = np.ascontiguousarray(
        wvT.reshape(HT, P, NKV * HD)).astype(bf)
    wqq = _quad_pack(wqT, 4, HT).astype(bf)
    woq = _quad_pack(woT, 4, NH).astype(bf)
    wgq = _quad_pack(wgT, FF // 512, HT).astype(bf)
    wuq = _quad_pack(wuT, FF // 512, HT).astype(bf)
    wdq = _quad_pack(wdT, 4, FC).astype(bf)
    return wkq, wvp, wqq, woq, wgq, wuq, wdq


def _mask_structure(m, S_):
    """Derive the global (ext, masked) structure from the [S, S] mask."""
    NT = S_ // TPG // P
    NB = S_ // 512
    col_of = np.empty((NB, 512), np.int64)
    for bi in range(NB):
        for o in range(TPG):
            col_of[bi, o * P:(o + 1) * P] = o + TPG * (P * bi + np.arange(P))
    need = np.zeros((NT, NB), bool)
    nonzero = np.zeros((NT, NB), bool)
    for r in range(TPG):
        for qt in range(NT):
            rows = r + TPG * (P * qt + np.arange(P))
            sub = m[rows]
            for bi in range(NB):
                blk = sub[:, col_of[bi]]
                need[qt, bi] |= bool((blk > MASK_CLAMP).any())
                nonzero[qt, bi] |= bool((blk < 0).any())
    ext = []
    masked = {}
    for qt in range(NT):
        e = int(np.max(np.nonzero(need[qt])[0])) + 1 if need[qt].any() else 1
        ext.append(e)
        for bi in range(e):
            if nonzero[qt, bi]:
                masked[(qt, bi)] = len(masked)
    return ext, masked, col_of


def kernel(hidden_states, attention_mask, q_w, k_w, v_w, o_w,
           gate_w, up_w, down_w, ln1_w, ln2_w):
    hidden_states = np.asarray(hidden_states, np.float32)
    m = np.maximum(np.asarray(attention_mask, np.float32)[0, 0], MASK_CLAMP)
    S_ = hidden_states.shape[1]
    FF_ = gate_w.shape[0]
    NT = S_ // TPG // P

    ext, masked, col_of = _mask_structure(m, S_)
    n_mask = len(masked)
    nc = _get_program(S_, FF_, ext, tuple(masked.items()), n_mask)

    wkq, wvp, wqq, woq, wgq, wuq, wdq = _prep_weights(
        np.asarray(q_w, np.float32), np.asarray(k_w, np.float32),
        np.asarray(v_w, np.float32), np.asarray(o_w, np.float32),
        np.asarray(gate_w, np.float32), np.asarray(up_w, np.float32),
        np.asarray(down_w, np.float32), np.asarray(ln1_w, np.float32),
        np.asarray(ln2_w, np.float32))

    in_maps = []
    for core in range(NC):
        b, r = core // TPG, core % TPG
        rows = r + TPG * np.arange(S_ // TPG)
        x_own = np.ascontiguousarray(
            hidden_states[b, rows].reshape(NT, P, H))
        mask_blocks = np.zeros((max(n_mask, 1), P, 512), np.float32)
        for (qt, bi), mi in masked.items():
            qrows = r + TPG * (P * qt + np.arange(P))
            mask_blocks[mi] = m[np.ix_(qrows, col_of[bi])]
        in_maps.append({
            "x": x_own, "mask": mask_blocks,
            "wkq": wkq, "wvp": wvp, "wqq": wqq, "woq": woq,
            "wgq": wgq, "wuq": wuq, "wdq": wdq,
        })

    res = run_bass_kernel_spmd(nc, in_maps, list(range(NC)),
                               trace=bool(os.environ.get("KERNEL_TRACE")))
    global LAST_RESULT
    LAST_RESULT = res

    out = np.empty((B, S_, H), np.float32)
    for core in range(NC):
        b, r = core // TPG, core % TPG
        rows = r + TPG * np.arange(S_ // TPG)
        out[b, rows] = res.results[core]["y"].reshape(S_ // TPG, H)
    return out


# revision 20
# speedup vs baseline: 1.1824x; 1.0838x over previous
"""Trainium2 Bass kernel for a dense GQA transformer layer (pre-norm, SwiGLU MLP).

Full shapes: B=2, S=2048, H=2048, NH=16, NKV=8, HD=128, FF=5632, fp32 I/O.

Sharding across 8 NeuronCores (one SPMD program):
  core = (b, r) with b = core//4 (data-parallel over batch),
  r = core%4 (sequence-parallel, row-interleaved: core owns rows r::4 of
  batch b). Row interleaving makes the causal-attention work identical on
  every core. K/V are computed for owned rows only and AllGather'ed
  (groups of 4). Everything else is token-parallel with full weights per
  core. Host reassembles the row-interleaved outputs.

v2 performance notes vs v1:
  - All 128x128 transposes moved off the PE onto the DMA XBAR
    (dma_start(transpose=True), 2-byte dtype).
  - Weight matmuls grouped in quads sharing one rhs stream; weights are
    host-packed so each quad is one contiguous 128KB DMA.
  - Q projection is weight-stationary and emits qT directly (no transpose).
  - Softmax normalization moved from ScalarE (ACT copy-scale pass) to
    VectorE tensor_scalar_mul.
  - K/V AllGathers kicked as soon as their halves are produced, overlapped
    with the rest of the QKV projections.
  - MLP/O weight DMAs issued from the Scalar engine queue, the rest from
    Sync, to spread HWDGE dispatch cost.

Precision: bf16 matmuls with fp32 PSUM accumulation; softmax, norms and
residuals in fp32. RMSNorm weights folded into the following projection
weights host-side; weights pre-transposed and tile-packed host-side.
"""

import sys

if "/opt/trn_rl_repo" not in sys.path:
    sys.path.insert(0, "/opt/trn_rl_repo")

import math
import os
import numpy as np
import ml_dtypes

import concourse.bass as bass
import concourse.bacc as bacc
import concourse.tile as tile
import concourse.mybir as mybir
from concourse.bass_utils import run_bass_kernel_spmd

F32 = mybir.dt.float32
BF16 = mybir.dt.bfloat16
AFT = mybir.ActivationFunctionType
ALU = mybir.AluOpType

# ---- fixed problem dims ----
B, S, H = 2, 2048, 2048
NH, NKV, HD = 16, 8, 128
FF = 5632
EPS = 1e-6
NC = 8          # cores
TPG = 4         # cores per batch group (sequence-parallel ways)
P = 128         # partitions

MASK_CLAMP = -30000.0


def _build_program(S_, FF_, ext, masked, n_mask):
    """Emit the SPMD program.

    S_: sequence length, FF_: mlp width
    ext: tuple, per q-tile number of 512-col key banks to compute
    masked: dict {(qt, bank): mask_slot_index} for banks needing a mask add
    n_mask: number of [128, 512] mask blocks in the mask input
    """
    TOWN = S_ // TPG              # tokens owned per core
    NT = TOWN // P                # q-tiles per core
    NB = S_ // 512                # key banks (512 cols each)
    HT = H // P                   # 16 H tiles
    FC = FF_ // P                 # 44 FF tiles
    FQ = FC // 4                  # 11 FF quads
    HQ = HT // 4                  # 4 H quads
    KVH = NKV
    assert len(ext) == NT

    nc = bacc.Bacc("TRN2", target_bir_lowering=False, debug=False,
                   num_devices=NC)

    # ---- I/O ----
    x_in = nc.dram_tensor("x", [NT, P, H], F32, kind="ExternalInput").ap()
    # packed weight tiles (see _prep_weights)
    wkq = nc.dram_tensor("wkq", [2, HT, P, 4, P], BF16, kind="ExternalInput").ap()
    wvp = nc.dram_tensor("wvp", [2, HT, P, 512], BF16, kind="ExternalInput").ap()
    wqq = nc.dram_tensor("wqq", [4, HT, P, 4, P], BF16, kind="ExternalInput").ap()
    woq = nc.dram_tensor("woq", [HQ, NH, P, 4, P], BF16, kind="ExternalInput").ap()
    wgq = nc.dram_tensor("wgq", [FQ, HT, P, 4, P], BF16, kind="ExternalInput").ap()
    wuq = nc.dram_tensor("wuq", [FQ, HT, P, 4, P], BF16, kind="ExternalInput").ap()
    wdq = nc.dram_tensor("wdq", [HQ, FC, P, 4, P], BF16, kind="ExternalInput").ap()
    mask_in = nc.dram_tensor("mask", [max(n_mask, 1), P, 512], F32,
                             kind="ExternalInput").ap()
    y_out = nc.dram_tensor("y", [NT, P, H], F32, kind="ExternalOutput").ap()

    # ---- internal DRAM for the K/V AllGather (split in halves) ----
    KH = KVH // 2                  # kv heads per half (4)
    k_loc = [nc.dram_tensor(f"k_loc{i}", [KH, HD, NT, P], BF16).ap()
             for i in range(2)]
    v_loc = [nc.dram_tensor(f"v_loc{i}", [NT, P, KH, HD], BF16).ap()
             for i in range(2)]
    k_all = [nc.dram_tensor(f"k_all{i}", [TPG, KH, HD, NT, P], BF16).ap()
             for i in range(2)]
    v_all = [nc.dram_tensor(f"v_all{i}", [TPG, NT, P, KH, HD], BF16).ap()
             for i in range(2)]

    groups = [[g * TPG + i for i in range(TPG)] for g in range(NC // TPG)]

    from contextlib import ExitStack
    with ExitStack() as ctx:
        tc = ctx.enter_context(tile.TileContext(nc))
        pool = lambda name, bufs, **kw: ctx.enter_context(
            tc.tile_pool(name=name, bufs=bufs, **kw))
        singles = pool("ones", 1)
        resid_pool = pool("resid", NT)
        ybuf = pool("ybuf", 1)          # Y then Y2 (same slot, rotated)
        scratch_pool = pool("scratch", 2)
        small_pool = pool("small", 8)
        wq4_pool = pool("wq4", 10)      # quad weight tiles [P,4,P]
        cpy_pool = pool("cpy", 4)
        xt_pool = pool("xtp", 2)        # transposed residual contributions
        ps_pool = pool("ps", 6, space="PSUM")      # [P,512] f32 accumulators
        pav_pool = pool("pav", 2, space="PSUM")    # AV accumulators

        # phase-1 (attention) scoped pools, released before the MLP pools
        ctx1 = ctx.enter_context(ExitStack())
        pool1 = lambda name, bufs, **kw: ctx1.enter_context(
            tc.tile_pool(name=name, bufs=bufs, **kw))
        qa_pool = pool1("qap", 1)
        aT_pool = pool1("aTp", 1)
        kv_pool = pool1("kvbuf", 2)
        pbf_pool = pool1("pbf", 6)
        pT_pool = pool1("pTp", 4)
        mask_pool = pool1("maskp", max(n_mask, 1))
        wv_pool = pool1("wvp_sb", 3)

        eps_c = singles.tile([P, 1], F32)
        nc.vector.memset(eps_c, EPS)

        # mask blocks (fp32, resident)
        mask_sb = []
        for mi in range(n_mask):
            mt = mask_pool.tile([P, 512], F32, tag="mask")
            nc.sync.dma_start(out=mt, in_=mask_in[mi])
            mask_sb.append(mt)

        def rmsnorm_to_ybf(xt):
            sq = scratch_pool.tile([P, H], BF16, tag="sq", bufs=1)
            ssum = small_pool.tile([P, 1], F32, tag="ss")
            nc.scalar.activation(out=sq, in_=xt, func=AFT.Square,
                                 accum_out=ssum)
            std = small_pool.tile([P, 1], F32, tag="std")
            nc.scalar.activation(out=std, in_=ssum, func=AFT.Sqrt,
                                 scale=1.0 / H, bias=eps_c)
            rstd = small_pool.tile([P, 1], F32, tag="rstd")
            nc.vector.reciprocal(rstd, std)
            ybf = scratch_pool.tile([P, H], BF16, tag="ybf")
            nc.scalar.activation(out=ybf, in_=xt, func=AFT.Copy, scale=rstd)
            return ybf

        # Y: [128 (h within tile), HT, TOWN] bf16, transposed activations
        def build_Y(x_tiles, name):
            Y = ybuf.tile([P, HT, TOWN], BF16, tag="y", name=name)
            for qt in range(NT):
                ybf = rmsnorm_to_ybf(x_tiles[qt])
                # one XBAR call: [tok,(ht h)] -> [h, ht, tok]
                nc.sync.dma_start(
                    out=Y[:, :, qt * P:(qt + 1) * P], in_=ybf,
                    transpose=True)
            return Y

        x_tiles = []
        for qt in range(NT):
            xt = resid_pool.tile([P, H], F32, tag="x")
            x_tiles.append(xt)
            nc.sync.dma_start(out=xt, in_=x_in[qt])
        Y = build_Y(x_tiles, "Y1")

        # ---- K projection (weight-stationary, quad-grouped) ----
        def k_half(kvq):
            pk = [ps_pool.tile([P, 512], F32, tag="mm", name=f"pk{kvq}_{j}")
                  for j in range(4)]
            for ht in range(HT):
                wl = wq4_pool.tile([P, 4, P], BF16, tag="w4")
                nc.scalar.dma_start(out=wl.rearrange("k a b -> k (a b)"),
                                    in_=wkq[kvq, ht].rearrange("k a b -> k (a b)"))
                for j in range(4):
                    nc.tensor.matmul(pk[j][:, :TOWN], lhsT=wl[:, j, :],
                                     rhs=Y[:, ht, :], start=(ht == 0),
                                     stop=(ht == HT - 1))
            for j in range(4):
                kc = cpy_pool.tile([P, 512], BF16, tag="kc")
                nc.vector.tensor_copy(kc[:, :TOWN], pk[j][:, :TOWN])
                nc.sync.dma_start(
                    out=k_loc[kvq][j].rearrange("d t i -> d (t i)"),
                    in_=kc[:, :TOWN])

        # ---- V projection half (Y-stationary): out [tok, 4*hd] ----
        def v_half(hf):
            for t in range(NT):
                pv = ps_pool.tile([P, 512], F32, tag="mm", name=f"pv{hf}_{t}")
                for ht in range(HT):
                    wv = wv_pool.tile([P, 512], BF16, tag="wv")
                    nc.scalar.dma_start(out=wv, in_=wvp[hf, ht])
                    nc.tensor.matmul(pv, lhsT=Y[:, ht, t * P:(t + 1) * P],
                                     rhs=wv, start=(ht == 0),
                                     stop=(ht == HT - 1))
                vc = cpy_pool.tile([P, 512], BF16, tag="kc")
                nc.vector.tensor_copy(vc, pv)
                nc.sync.dma_start(
                    out=v_loc[hf][t].rearrange("s k d -> s (k d)"), in_=vc)

        def gather(loc, al):
            nc.gpsimd.collective_compute(
                "AllGather", ALU.bypass, ins=[loc.opt()],
                outs=[al.opt()], replica_groups=groups)

        k_half(0)
        gather(k_loc[0], k_all[0])
        v_half(0)
        gather(v_loc[0], v_all[0])
        k_half(1)
        gather(k_loc[1], k_all[1])
        v_half(1)
        gather(v_loc[1], v_all[1])

        # ---- Q projection (weight-stationary -> qT directly) ----
        qT = qa_pool.tile([P, NH, TOWN], BF16, tag="qT")
        for qq in range(4):
            pq = [ps_pool.tile([P, 512], F32, tag="mm", name=f"pq{qq}_{j}")
                  for j in range(4)]
            for ht in range(HT):
                wl = wq4_pool.tile([P, 4, P], BF16, tag="w4")
                nc.scalar.dma_start(out=wl.rearrange("k a b -> k (a b)"),
                                    in_=wqq[qq, ht].rearrange("k a b -> k (a b)"))
                for j in range(4):
                    nc.tensor.matmul(pq[j][:, :TOWN], lhsT=wl[:, j, :],
                                     rhs=Y[:, ht, :], start=(ht == 0),
                                     stop=(ht == HT - 1))
            for j in range(4):
                nc.vector.tensor_copy(qT[:, qq * 4 + j, :],
                                      pq[j][:, :TOWN])

        # ---- attention ----
        aT = aT_pool.tile([P, NH, TOWN], BF16, tag="aT")
        for kvh in range(KVH):
            kT_sb = kv_pool.tile([P, NB, TPG, P], BF16, tag="kT")
            v_sb = kv_pool.tile([P, NB, TPG, HD], BF16, tag="vT")
            ka, va = k_all[kvh // KH], v_all[kvh // KH]
            for o in range(TPG):
                nc.sync.dma_start(out=kT_sb[:, :, o, :], in_=ka[o, kvh % KH])
                nc.sync.dma_start(
                    out=v_sb[:, :, o, :],
                    in_=va[o].rearrange("t s k d -> s t k d")[:, :, kvh % KH, :])
            for qt in range(NT):
                nbank = ext[qt]
                # pT tile: [key, pc, h2, tok] bf16
                pTt = pT_pool.tile([P, NB * TPG, 2, P], BF16, tag="pT")
                for h2 in range(2):
                    h = 2 * kvh + h2
                    accs = small_pool.tile([P, NB], F32, tag="accs")
                    pb = pbf_pool.tile([P, NB * 512], BF16, tag="pb")
                    for bi in range(nbank):
                        psc = ps_pool.tile([P, 512], F32, tag="mm")
                        nc.tensor.matmul(
                            psc, lhsT=qT[:, h, qt * P:(qt + 1) * P],
                            rhs=kT_sb[:, bi, :, :].rearrange(
                                "d o i -> d (o i)"),
                            start=True, stop=True)
                        mi = masked.get((qt, bi))
                        if mi is not None:
                            nc.vector.tensor_add(psc, psc, mask_sb[mi])
                        nc.scalar.activation(
                            out=pb[:, bi * 512:(bi + 1) * 512], in_=psc,
                            func=AFT.Exp, accum_out=accs[:, bi:bi + 1])
                    den = small_pool.tile([P, 1], F32, tag="den")
                    nc.vector.tensor_reduce(den, accs[:, :nbank],
                                            mybir.AxisListType.X, ALU.add)
                    rec = small_pool.tile([P, 1], F32, tag="rec")
                    nc.vector.reciprocal(rec, den)
                    nc.vector.tensor_scalar_mul(pb[:, :nbank * 512],
                                                pb[:, :nbank * 512], rec)
                    # XBAR: [tok, (pc key)] -> [key, pc, tok]
                    nc.sync.dma_start(
                        out=pTt[:, :nbank * TPG, h2, :],
                        in_=pb[:, :nbank * 512], transpose=True)
                pav = pav_pool.tile([P, 2, P], F32, tag="av",
                                    padded_shape=[P, 2, 256])
                for pc in range(nbank * TPG):
                    nc.tensor.matmul(
                        pav, lhsT=v_sb[:, pc // TPG, pc % TPG, :],
                        rhs=pTt[:, pc, :, :].rearrange("s h i -> s (h i)"),
                        start=(pc == 0), stop=(pc == nbank * TPG - 1))
                nc.vector.tensor_copy(
                    aT[:, 2 * kvh:2 * kvh + 2, qt * P:(qt + 1) * P], pav)

        # ---- O projection + residual ----
        def out_proj_pass(w_dram, rhs_fn, n_acc, hq, dma_eng):
            """One quad accumulation pass producing [Hcol(4x128), TOWN] f32."""
            po = [ps_pool.tile([P, 512], F32, tag="mm", name=f"po{hq}_{j}")
                  for j in range(4)]
            for a in range(n_acc):
                wl = wq4_pool.tile([P, 4, P], BF16, tag="w4")
                dma_eng.dma_start(
                    out=wl.rearrange("k a b -> k (a b)"),
                    in_=w_dram[hq, a].rearrange("k a b -> k (a b)"))
                rhs = rhs_fn(a)
                for j in range(4):
                    nc.tensor.matmul(po[j][:, :TOWN], lhsT=wl[:, j, :],
                                     rhs=rhs, start=(a == 0),
                                     stop=(a == n_acc - 1))
            # transpose back into residual: per j one XBAR + NT adds
            for j in range(4):
                dc = cpy_pool.tile([P, 512], BF16, tag="kc")
                nc.vector.tensor_copy(dc[:, :TOWN], po[j][:, :TOWN])
                xT = xt_pool.tile([P, NT, P], BF16, tag="xT")
                nc.sync.dma_start(out=xT[:, :NT, :], in_=dc[:, :TOWN],
                                  transpose=True)
                hc = hq * 4 + j
                for qt in range(NT):
                    xb = x_tiles[qt][:, hc * P:(hc + 1) * P]
                    nc.vector.tensor_add(xb, xb, xT[:, qt, :])

        for hq in range(HQ):
            out_proj_pass(woq, lambda h: aT[:, h, :], NH, hq, nc.scalar)

        # release attention pools, open MLP pools
        ctx1.close()
        mT_pool = pool("mTp", FC)

        # ---- rmsnorm2 -> Y2 ----
        Y2 = build_Y(x_tiles, "Y2")

        # ---- MLP gate/up ----
        mT = []
        for fq in range(FQ):
            pg = [ps_pool.tile([P, 512], F32, tag="mm", name=f"pg{fq}_{j}")
                  for j in range(4)]
            for ht in range(HT):
                wl = wq4_pool.tile([P, 4, P], BF16, tag="w4")
                nc.sync.dma_start(
                    out=wl.rearrange("k a b -> k (a b)"),
                    in_=wgq[fq, ht].rearrange("k a b -> k (a b)"))
                for j in range(4):
                    nc.tensor.matmul(pg[j][:, :TOWN], lhsT=wl[:, j, :],
                                     rhs=Y2[:, ht, :], start=(ht == 0),
                                     stop=(ht == HT - 1))
            sg = [cpy_pool.tile([P, 512], BF16, tag="kc", name=f"sg{fq}_{j}")
                  for j in range(4)]
            for j in range(4):
                nc.scalar.activation(out=sg[j][:, :TOWN], in_=pg[j][:, :TOWN],
                                     func=AFT.Silu)
            pu = [ps_pool.tile([P, 512], F32, tag="mm", name=f"pu{fq}_{j}")
                  for j in range(4)]
            for ht in range(HT):
                wl = wq4_pool.tile([P, 4, P], BF16, tag="w4")
                nc.sync.dma_start(
                    out=wl.rearrange("k a b -> k (a b)"),
                    in_=wuq[fq, ht].rearrange("k a b -> k (a b)"))
                for j in range(4):
                    nc.tensor.matmul(pu[j][:, :TOWN], lhsT=wl[:, j, :],
                                     rhs=Y2[:, ht, :], start=(ht == 0),
                                     stop=(ht == HT - 1))
            for j in range(4):
                mt = mT_pool.tile([P, TOWN], BF16, tag="mT")
                mT.append(mt)
                nc.vector.tensor_mul(mt, sg[j][:, :TOWN], pu[j][:, :TOWN])

        # ---- MLP down + residual ----
        for hq in range(HQ):
            out_proj_pass(wdq, lambda fc: mT[fc], FC, hq, nc.sync)

        for qt in range(NT):
            nc.sync.dma_start(out=y_out[qt], in_=x_tiles[qt])

    nc.compile()
    return nc


_CACHE = {}
LAST_RESULT = None


def _get_program(S_, FF_, ext, masked_items, n_mask):
    key = (S_, FF_, tuple(ext), tuple(sorted(masked_items)), n_mask)
    if key not in _CACHE:
        _CACHE[key] = _build_program(S_, FF_, tuple(ext), dict(masked_items),
                                     n_mask)
    return _CACHE[key]


def _quad_pack(wT, n_outer, inner_tiles):
    """[K, N] -> [n_outer, inner_tiles, 128, 4, 128] quad tiles.

    outer o covers output cols o*512..(o+1)*512, inner a covers input rows
    a*128..(a+1)*128.
    """
    K, N = wT.shape
    assert K == inner_tiles * P and N == n_outer * 512
    w = wT.reshape(inner_tiles, P, n_outer, 4, P)
    return np.ascontiguousarray(w.transpose(2, 0, 1, 3, 4))


def _prep_weights(q_w, k_w, v_w, o_w, gate_w, up_w, down_w, ln1_w, ln2_w):
    bf = ml_dtypes.bfloat16
    HT = H // P
    FC = FF // P
    wqT = (q_w * ln1_w[None, :]).T * (1.0 / math.sqrt(HD))   # [H, NH*HD]
    wkT = (k_w * ln1_w[None, :]).T                           # [H, NKV*HD]
    wvT = (v_w * ln1_w[None, :]).T
    woT = o_w.T                                              # [NH*HD, H]
    wgT = (gate_w * ln2_w[None, :]).T                        # [H, FF]
    wuT = (up_w * ln2_w[None, :]).T
    wdT = down_w.T                                           # [FF, H]
    wkq = _quad_pack(wkT, 2, HT).astype(bf)
    wvp = np.ascontiguousarray(
        wvT.reshape(HT, P, 2, 512).transpose(2, 0, 1, 3)).astype(bf)
    wqq = _quad_pack(wqT, 4, HT).astype(bf)
    woq = _quad_pack(woT, 4, NH).astype(bf)
    wgq = _quad_pack(wgT, FF // 512, HT).astype(bf)
    wuq = _quad_pack(wuT, FF // 512, HT).astype(bf)
    wdq = _quad_pack(wdT, 4, FC).astype(bf)
    return wkq, wvp, wqq, woq, wgq, wuq, wdq


def _mask_structure(m, S_):
    """Derive the global (ext, masked) structure from the [S, S] mask."""
    NT = S_ // TPG // P
    NB = S_ // 512
    col_of = np.empty((NB, 512), np.int64)
    for bi in range(NB):
        for o in range(TPG):
            col_of[bi, o * P:(o + 1) * P] = o + TPG * (P * bi + np.arange(P))
    need = np.zeros((NT, NB), bool)
    nonzero = np.zeros((NT, NB), bool)
    for r in range(TPG):
        for qt in range(NT):
            rows = r + TPG * (P * qt + np.arange(P))
            sub = m[rows]
            for bi in range(NB):
                blk = sub[:, col_of[bi]]
                need[qt, bi] |= bool((blk > MASK_CLAMP).any())
                nonzero[qt, bi] |= bool((blk < 0).any())
    ext = []
    masked = {}
    for qt in range(NT):
        e = int(np.max(np.nonzero(need[qt])[0])) + 1 if need[qt].any() else 1
        ext.append(e)
        for bi in range(e):
            if nonzero[qt, bi]:
                masked[(qt, bi)] = len(masked)
    return ext, masked, col_of


def kernel(hidden_states, attention_mask, q_w, k_w, v_w, o_w,
           gate_w, up_w, down_w, ln1_w, ln2_w):
    hidden_states = np.asarray(hidden_states, np.float32)
    m = np.maximum(np.asarray(attention_mask, np.float32)[0, 0], MASK_CLAMP)
    S_ = hidden_states.shape[1]
    FF_ = gate_w.shape[0]
    NT = S_ // TPG // P

    ext, masked, col_of = _mask_structure(m, S_)
    n_mask = len(masked)
    nc = _get_program(S_, FF_, ext, tuple(masked.items()), n_mask)

    wkq, wvp, wqq, woq, wgq, wuq, wdq = _prep_weights(
        np.asarray(q_w, np.float32), np.asarray(k_w, np.float32),
        np.asarray(v_w, np.float32), np.asarray(o_w, np.float32),
        np.asarray(gate_w, np.float32), np.asarray(up_w, np.float32),
        np.asarray(down_w, np.float32), np.asarray(ln1_w, np.float32),
        np.asarray(ln2_w, np.float32))

    in_maps = []
    for core in range(NC):
        b, r = core // TPG, core % TPG
        rows = r + TPG * np.arange(S_ // TPG)
        x_own = np.ascontiguousarray(
            hidden_states[b, rows].reshape(NT, P, H))
        mask_blocks = np.zeros((max(n_mask, 1), P, 512), np.float32)
        for (qt, bi), mi in masked.items():
            qrows = r + TPG * (P * qt + np.arange(P))
            mask_blocks[mi] = m[np.ix_(qrows, col_of[bi])]
        in_maps.append({
            "x": x_own, "mask": mask_blocks,
            "wkq": wkq, "wvp": wvp, "wqq": wqq, "woq": woq,
            "wgq": wgq, "wuq": wuq, "wdq": wdq,
        })

    res = run_bass_kernel_spmd(nc, in_maps, list(range(NC)),
                               trace=bool(os.environ.get("KERNEL_TRACE")))
    global LAST_RESULT
    LAST_RESULT = res

    out = np.empty((B, S_, H), np.float32)
    for core in range(NC):
        b, r = core // TPG, core % TPG
        rows = r + TPG * np.arange(S_ // TPG)
        out[b, rows] = res.results[core]["y"].reshape(S_ // TPG, H)
    return out
